# revision 28
# baseline (speedup 1.0000x reference)
"""Gemma3 decoder layer on 8 Trainium2 NeuronCores (Bass/Tile), v2.

Sharding (per core c):
  - x^T replicated to every core (host-side); in_ln folded into wq/wk/wv on
    the host: the per-token 1/rms cancels through the q/k head RMSNorms, and
    V is rescaled by rinv computed on-device (xsq-matmul against ones).
    => no activation AllGather before attention.
  - attention: tensor-parallel over heads; core c owns Q head c, KV head
    c//2, computed for the FULL sequence straight from x^T.
  - wo row-sharded: each core computes its head's full-sequence partial
    attn_head @ wo[head rows]; two striped ReduceScatters (one per 128-token
    half of each shard) overlap with attention/wo compute.
  - MLP gate/up column-sharded, down row-sharded. h2^T is AllGathered in 4
    uneven hid-chunks (3/4/6/7 of 20) consumed by SBUF-accumulated partial
    matmuls so the gather pipelines with gate/up compute. down partials are
    reduced with two striped ReduceScatters.
Matmuls in bf16 (fp32 PSUM); norms/softmax/residual fp32.
"""
import sys

if "/opt/trn_rl_repo" not in sys.path:
    sys.path.insert(0, "/opt/trn_rl_repo")

import numpy as np
import ml_dtypes

import concourse.bass as bass
import concourse.mybir as mybir
import concourse.tile as tile
from concourse import bacc
from concourse.bass_utils import run_bass_kernel_spmd
from concourse.masks import make_identity

dt = mybir.dt
AF = mybir.ActivationFunctionType
ALU = mybir.AluOpType
BF = dt.bfloat16
F32 = dt.float32

HID, NH, NKV, HD, INTER = 2560, 8, 4, 256, 10240
WIN, EPS, BASE = 512, 1e-6, 10000.0
S = 2048
NC_ = 8
TS = S // NC_              # 256 tokens per core
KH = HID // 128            # 20 hidden-dim chunks
MI = INTER // NC_ // 128   # 10 inter m-tiles per core
HALF = HD // 2
KSPLIT = [0, 4, 8, 12, 16, 20]  # h2T AG chunk boundaries (128-row k units)
NG = len(KSPLIT) - 1
NT = S // 128              # 16 token tiles


def _bcast_row(nc, sbuf_tile, dram_t, width):
    a = dram_t.ap()
    nc.sync.dma_start(sbuf_tile[:], bass.AP(
        tensor=a.tensor, offset=a.offset, ap=[[0, 128], [1, width]]))


def build_nc(sim=False, core_id=0):
    nc = bacc.Bacc("TRN2", target_bir_lowering=False, debug=False,
                   enable_asserts=True, num_devices=1 if sim else NC_)

    def _coll(kind, op, ins, outs):
        if not sim:
            nc.gpsimd.collective_compute(kind, op, replica_groups=rg,
                                         ins=ins, outs=outs)
            return
        i_ap, o_ap = ins[0], outs[0]
        if kind == "AllGather":
            n = i_ap.shape[0]
            for r in range(NC_):
                nc.sync.dma_start(o_ap[r * n:(r + 1) * n], i_ap)
        elif kind == "ReduceScatter":
            n = o_ap.shape[0]
            nc.sync.dma_start(o_ap, i_ap[core_id * n:(core_id + 1) * n])

    x_shard = nc.dram_tensor("x_shard", [TS, HID], F32, kind="ExternalInput")
    xT4 = nc.dram_tensor("xT4", [4, HID, S // 4], BF, kind="ExternalInput")
    wqk_c = nc.dram_tensor("wqk_c", [HID, 2 * HD], BF, kind="ExternalInput")
    wv_c = nc.dram_tensor("wv_c", [HID, HD], BF, kind="ExternalInput")
    wo_c = nc.dram_tensor("wo_c", [HD, HID], BF, kind="ExternalInput")
    wg_t = nc.dram_tensor("wg_t", [KH, 128, MI * 128], BF, kind="ExternalInput")
    wu_t = nc.dram_tensor("wu_t", [KH, 128, MI * 128], BF, kind="ExternalInput")
    wd_t = nc.dram_tensor("wd_t", [5, MI, 128, 512], BF, kind="ExternalInput")
    w1_pa = nc.dram_tensor("w1_pa", [HID], BF, kind="ExternalInput")
    w1_pf = nc.dram_tensor("w1_pf", [HID], BF, kind="ExternalInput")
    w1_po = nc.dram_tensor("w1_po", [HID], F32, kind="ExternalInput")
    cqw = nc.dram_tensor("cqw", [S, HD], BF, kind="ExternalInput")
    sqw = nc.dram_tensor("sqw", [S, HD], BF, kind="ExternalInput")
    ckw = nc.dram_tensor("ckw", [S, HD], BF, kind="ExternalInput")
    skw = nc.dram_tensor("skw", [S, HD], BF, kind="ExternalInput")
    out_shard = nc.dram_tensor("out_shard", [TS, HID], F32, kind="ExternalOutput")

    rg = [list(range(NC_))]
    stages = {}
    nc._stage_ids = stages

    def mark(name):
        stages[name] = nc.next_id()

    with tile.TileContext(nc) as tc:
        with (
            tc.tile_pool(name="dram", bufs=1, space="DRAM") as dram,
            tc.tile_pool(name="glob", bufs=1) as glob,
            tc.tile_pool(name="nrm", bufs=3) as nrm,
            tc.tile_pool(name="psP", bufs=1, space="PSUM") as psP,
        ):
            # DRAM scratch
            rs_in = [dram.tile([NC_ * 128, HID], BF, name=f"rs_in{j}")
                     for j in range(2)]
            rs_out = [dram.tile([128, HID], BF, name=f"rs_out{j}")
                      for j in range(2)]
            h2T_in = dram.tile([HID, TS], BF)
            h2T_full = [
                dram.tile([NC_ * (KSPLIT[g + 1] - KSPLIT[g]) * 128, TS], BF,
                          addr_space="Local" if sim else "Shared",
                          name=f"h2T_full{g}")
                for g in range(NG)]
            rs2_in = [dram.tile([NC_ * 128, HID], BF, name=f"rs2_in{j}")
                      for j in range(2)]
            rs2_out = [dram.tile([128, HID], BF, name=f"rs2_out{j}")
                       for j in range(2)]

            ident = glob.tile([128, 128], BF)
            make_identity(nc, ident[:])
            eps_t = glob.tile([128, 1], F32)
            nc.vector.memset(eps_t[:], EPS)
            rinv_sb = glob.tile([128, NT], F32)

            def rmsnorm_rinv_act(src_ap, d, name, scratch):
                """Act-engine variant: Square w/ accum_out -> sumsq."""
                ss = nrm.tile([128, 1], F32, tag="nss", name=f"{name}_ss")
                nc.scalar.activation(scratch, src_ap, AF.Square,
                                     accum_out=ss[:])
                sq = nrm.tile([128, 1], F32, tag="nln", name=f"{name}_sq")
                nc.scalar.activation(sq[:], ss[:], AF.Sqrt, bias=eps_t[:],
                                     scale=1.0 / d)
                rinv = nrm.tile([128, 1], F32, tag="nrv", name=f"{name}_rv")
                nc.vector.reciprocal(rinv[:], sq[:])
                return rinv

            def rmsnorm_rinv(src_ap, d, name):
                """rinv[p,1]=1/sqrt(mean(src^2)+EPS) via bn_stats + ln/exp."""
                nsub = max(1, d // 512)
                stats = nrm.tile([128, nsub, 6], F32, tag="nst", name=f"{name}_st")
                if nsub > 1:
                    view = src_ap.rearrange("p (s f) -> p s f", s=nsub)
                    for i in range(nsub):
                        nc.vector.bn_stats(out=stats[:, i, :], in_=view[:, i, :])
                else:
                    nc.vector.bn_stats(out=stats[:, 0, :], in_=src_ap)
                mv = nrm.tile([128, 2], F32, tag="nmv", name=f"{name}_mv")
                nc.vector.bn_aggr(out=mv[:], in_=stats[:])
                ms = nrm.tile([128, 1], F32, tag="nms", name=f"{name}_ms")
                nc.vector.scalar_tensor_tensor(ms[:], mv[:, 0:1], mv[:, 0:1],
                                               mv[:, 1:2], op0=ALU.mult, op1=ALU.add)
                sq = nrm.tile([128, 1], F32, tag="nln", name=f"{name}_sq")
                nc.scalar.activation(sq[:], ms[:], AF.Sqrt, bias=eps_t[:])
                rinv = nrm.tile([128, 1], F32, tag="nrv", name=f"{name}_rv")
                nc.vector.reciprocal(rinv[:], sq[:])
                return rinv

            # x2 lives S4..S6 (opened first for LIFO pool order);
            # x_sb lives S2..S4.
            x2pool_cm = tc.tile_pool(name="x2pool", bufs=1)
            x2pool = x2pool_cm.__enter__()
            x2_sb = [x2pool.tile([128, HID], F32, name=f"x2t{t}")
                     for t in range(2)]
            xpool_cm = tc.tile_pool(name="xpool", bufs=1)
            xpool = xpool_cm.__enter__()
            x_sb = [xpool.tile([128, HID], F32, name=f"xt{t}") for t in range(2)]

            # Attention-lifetime SBUF (QT/KT/V/masks survive S2+S3)
            attp_cm = tc.tile_pool(name="attp", bufs=1)
            attp = attp_cm.__enter__()
            QTm = attp.tile([128, 2, S], BF, name="QTm")
            KTm = attp.tile([128, 2, S], BF, name="KTm")
            V = [attp.tile([128, HD + 1], BF, name=f"V{i}") for i in range(NT)]
            for i in range(NT):
                nc.vector.memset(V[i][:, HD:HD + 1], 1.0)
            masks = attp.tile([128, 8, 512], BF)
            for i in range(8):
                delta = 512 - 128 * i
                mk = masks[:, i, :]
                nc.gpsimd.memset(mk, 1.0)
                nc.gpsimd.affine_select(
                    out=mk, in_=mk, compare_op=ALU.is_ge, fill=0.0,
                    base=delta, pattern=[[1, 512]], channel_multiplier=-1)
                nc.gpsimd.affine_select(
                    out=mk, in_=mk, compare_op=ALU.is_ge, fill=0.0,
                    base=-delta + (WIN - 1), pattern=[[-1, 512]],
                    channel_multiplier=1)

            mark('S2')
            # ============ S2: QKV projections for full S from x^T ============
            with tc.tile_pool(name="s2", bufs=2) as s2:
                wqk_sb = s2.tile([128, KH, 2 * HD], BF, bufs=1)
                nc.sync.dma_start(wqk_sb[:],
                                  wqk_c.ap().rearrange("(k p) n -> p k n", p=128))
                wv_sb = s2.tile([128, KH, HD], BF, bufs=1)
                nc.sync.dma_start(wv_sb[:],
                                  wv_c.ap().rearrange("(k p) n -> p k n", p=128))
                xT_sb = s2.tile([128, KH, S], BF, bufs=1)
                xT_v = xT_sb[:].rearrange("p k (q t) -> p k q t", q=4)
                for q in range(4):
                    nc.sync.dma_start(
                        xT_v[:, :, q, :],
                        xT4.ap()[q].rearrange("(k p) t -> p k t", p=128))
                for t in range(2):
                    nc.sync.dma_start(x_sb[t][:],
                                      x_shard.ap()[t * 128:(t + 1) * 128, :])
                tabs = {"cq": cqw, "sq": sqw, "ck": ckw, "sk": skw}

                def qkv_mm(tt):
                    pqk = psP.tile([128, 2 * HD], F32, tag="mm", bufs=6,
                                   name=f"pqk{tt}")
                    pv = psP.tile([128, HD], F32, tag="mm", bufs=6,
                                  name=f"pv{tt}")
                    for k in range(KH):
                        st, sp = (k == 0), (k == KH - 1)
                        lh = xT_sb[:, k, tt * 128:(tt + 1) * 128]
                        nc.tensor.matmul(pqk[:], lh, wqk_sb[:, k, :],
                                         start=st, stop=sp)
                        nc.tensor.matmul(pv[:], lh, wv_sb[:, k, :],
                                         start=st, stop=sp)
                    return pqk, pv

                def qkv_post(tt, pqk, pv):
                    nc.scalar.activation(V[tt][:, 0:HD], pv[:], AF.Copy)
                    for (qo, cnm, snm, QKT, nm) in (
                            (0, "cq", "sq", QTm, "q"),
                            (HD, "ck", "sk", KTm, "k")):
                        srcp = pqk[:, qo:qo + HD]
                        swp = bass.AP(tensor=srcp.tensor,
                                      offset=srcp.offset + HALF,
                                      ap=[list(srcp.ap[0]),
                                          [-HALF, 2], [1, HALF]])
                        rinv = rmsnorm_rinv(srcp, HD, f"{nm}n{tt}")
                        ct = s2.tile([128, HD], BF, tag=f"tab{cnm}",
                                     name=f"{cnm}{tt}", bufs=2)
                        st_ = s2.tile([128, HD], BF, tag=f"tab{snm}",
                                      name=f"{snm}{tt}", bufs=2)
                        nc.sync.dma_start(
                            ct[:], tabs[cnm].ap()[tt * 128:(tt + 1) * 128, :])
                        nc.sync.dma_start(
                            st_[:], tabs[snm].ap()[tt * 128:(tt + 1) * 128, :])
                        t1 = s2.tile([128, HD], BF, tag="t1",
                                     name=f"t1{nm}{tt}", bufs=2)
                        t2 = s2.tile([128, HD], BF, tag="t2",
                                     name=f"t2{nm}{tt}", bufs=2)
                        nc.vector.scalar_tensor_tensor(
                            t1[:], srcp, rinv[:], ct[:],
                            op0=ALU.mult, op1=ALU.mult)
                        nc.vector.scalar_tensor_tensor(
                            t2[:].rearrange("p (a b) -> p a b", a=2),
                            swp, rinv[:],
                            st_[:].rearrange("p (a b) -> p a b", a=2),
                            op0=ALU.mult, op1=ALU.mult)
                        qr = s2.tile([128, HD], BF, tag="qr",
                                     name=f"qr{nm}{tt}", bufs=2)
                        nc.gpsimd.tensor_add(qr[:], t1[:], t2[:])
                        ptr = psP.tile([128, HD], BF, tag="tr",
                                       bufs=2, name=f"s2t{nm}{tt}")
                        for h in range(2):
                            nc.tensor.transpose(
                                ptr[:, h * 128:(h + 1) * 128],
                                qr[:, h * 128:(h + 1) * 128], ident[:])
                        if nm == "q":
                            nc.vector.tensor_copy(
                                QKT[:, :, tt * 128:(tt + 1) * 128],
                                ptr[:].rearrange("p (a b) -> p a b", a=2))
                        else:
                            nc.scalar.activation(
                                QKT[:, :, tt * 128:(tt + 1) * 128],
                                ptr[:].rearrange("p (a b) -> p a b", a=2),
                                AF.Copy)

                # software pipeline: matmuls for tt, postprocess for tt-2;
                # in_ln rinv (for V-rescale) interleaved: xsq squares on the
                # idle Pool engine, tiny pss column-matmuls between the tt
                # projection matmuls.
                ones_t = s2.tile([128, 1], BF, bufs=1)
                nc.vector.memset(ones_t[:], 1.0)
                pss = psP.tile([128, NT], F32, tag="tr", bufs=2, name="pss")

                def rinv_mm(tt):
                    q, ts_ = tt // 4, tt % 4
                    xsq = s2.tile([128, KH, 128], BF, tag="xsq",
                                  name=f"xsq{tt}", bufs=3)
                    nc.gpsimd.tensor_mul(
                        xsq[:],
                        xT_v[:, :, q, ts_ * 128:(ts_ + 1) * 128],
                        xT_v[:, :, q, ts_ * 128:(ts_ + 1) * 128])
                    for k in range(KH):
                        nc.tensor.matmul(
                            pss[:, tt:tt + 1], xsq[:, k, :], ones_t[:],
                            start=(k == 0), stop=(k == KH - 1))

                pend = {}
                for tt in range(NT):
                    pend[tt] = qkv_mm(tt)
                    rinv_mm(tt)
                    if tt - 2 in pend:
                        qkv_post(tt - 2, *pend.pop(tt - 2))
                for tt in sorted(pend):
                    qkv_post(tt, *pend.pop(tt))
                ssq = s2.tile([128, NT], F32, bufs=1)
                nc.scalar.activation(ssq[:], pss[:], AF.Sqrt, bias=eps_t[:],
                                     scale=1.0 / HID)
                nc.vector.reciprocal(rinv_sb[:], ssq[:])

            mark('S3')
            # ====== S3: attention + row-sharded wo partial + striped RS ======
            with tc.tile_pool(name="s3", bufs=2) as s3:
                wo_sb = s3.tile([128, 2, HID], BF, bufs=1)
                nc.sync.dma_start(wo_sb[:],
                                  wo_c.ap().rearrange("(h p) n -> p h n", p=128))
                # rescale V rows by in_ln rinv (tokens on partitions)
                for tt in range(NT):
                    nc.gpsimd.tensor_scalar_mul(
                        V[tt][:, 0:HD], V[tt][:, 0:HD], rinv_sb[:, tt:tt + 1])
                probs = {}

                def scores(qb):
                    q0 = 512 * qb
                    for i in range(8):
                        kc = q0 - 512 + 128 * i
                        if kc < 0:
                            continue
                        psc = psP.tile([128, 512], F32, tag="mm", bufs=6,
                                       name=f"psc{qb}_{i}")
                        for h in range(2):
                            nc.tensor.matmul(psc[:], KTm[:, h, kc:kc + 128],
                                             QTm[:, h, q0:q0 + 512],
                                             start=(h == 0), stop=(h == 1))
                        pr = s3.tile([128, 512], BF, name=f"pr{qb}_{i}", bufs=1)
                        nc.scalar.activation(pr[:], psc[:], AF.Exp,
                                             scale=1.0 / 16.0)
                        nc.vector.tensor_mul(pr[:], pr[:], masks[:, i, :])
                        probs[(qb, kc)] = pr

                def pv_mm(tt):
                    qb, qs = tt // 4, tt % 4
                    qa = 512 * qb + 128 * qs
                    kcs = [kc for kc in range(qa - 512, qa + 128, 128)
                           if kc >= 0]
                    po = psP.tile([128, HD + 1], F32, tag="mm", bufs=6,
                                  name=f"po{tt}")
                    col = 128 * qs
                    for j, kc in enumerate(kcs):
                        nc.tensor.matmul(po[:],
                                         probs[(tt // 4, kc)][:, col:col + 128],
                                         V[kc // 128][:], start=(j == 0),
                                         stop=(j == len(kcs) - 1))
                    return po

                def pv_post(tt, po):
                    rec = s3.tile([128, 1], F32, tag="rec", name=f"rec{tt}")
                    nc.vector.reciprocal(rec[:], po[:, HD:HD + 1])
                    an = s3.tile([128, HD], BF, tag="an", name=f"an{tt}")
                    nc.vector.tensor_scalar_mul(an[:], po[:, 0:HD], rec[:])
                    ptr = psP.tile([128, HD], BF, tag="tr", bufs=2,
                                   name=f"s3tr{tt}")
                    for h in range(2):
                        nc.tensor.transpose(
                            ptr[:, h * 128:(h + 1) * 128],
                            an[:, h * 128:(h + 1) * 128], ident[:])
                    aT = s3.tile([128, 2, 128], BF, tag="aT", name=f"aT{tt}",
                                 bufs=3)
                    if tt % 2 == 0:
                        nc.vector.tensor_copy(
                            aT[:], ptr[:].rearrange("p (a b) -> p a b", a=2))
                    else:
                        nc.scalar.activation(
                            aT[:], ptr[:].rearrange("p (a b) -> p a b", a=2),
                            AF.Copy)
                    # wo partial: [128 tok, HID] = aT.T @ wo_c
                    wop = s3.tile([128, HID], BF, tag="wop", name=f"wop{tt}",
                                  bufs=3)
                    for nn_ in range(5):
                        pw = psP.tile([128, 512], F32, tag="mm", bufs=6,
                                      name=f"pw{tt}_{nn_}")
                        for h in range(2):
                            nc.tensor.matmul(
                                pw[:], aT[:, h, :],
                                wo_sb[:, h, nn_ * 512:(nn_ + 1) * 512],
                                start=(h == 0), stop=(h == 1))
                        sl = slice(nn_ * 512, (nn_ + 1) * 512)
                        if nn_ % 2 == 0:
                            nc.vector.tensor_copy(wop[:, sl], pw[:])
                        else:
                            nc.scalar.activation(wop[:, sl], pw[:], AF.Copy)
                    j, blk = tt % 2, tt // 2
                    nc.sync.dma_start(
                        rs_in[j][:][blk * 128:(blk + 1) * 128, :], wop[:])

                # even token tiles first so stripe-0's ReduceScatter can
                # launch while the odd tiles compute; scores per 512-block on
                # first touch; PV postprocess software-pipelined.
                pend = {}
                done_qb = set()

                def run_tt(tt, pdepth=2):
                    qb = tt // 4
                    if qb not in done_qb:
                        done_qb.add(qb)
                        scores(qb)
                    pend[tt] = pv_mm(tt)
                    ks = [k for k in pend]
                    if len(ks) > pdepth:
                        pv_post(ks[0], pend.pop(ks[0]))

                def flush():
                    for tt in list(pend):
                        pv_post(tt, pend.pop(tt))

                for tt in range(0, NT, 2):
                    run_tt(tt)
                flush()
                _coll("ReduceScatter", ALU.add, [rs_in[0][:]], [rs_out[0][:]])
                for tt in range(1, NT, 2):
                    run_tt(tt)
                flush()
                _coll("ReduceScatter", ALU.add, [rs_in[1][:]], [rs_out[1][:]])
            attp_cm.__exit__(None, None, None)

            mark('S4')
            # ==== S4: post_attn norm + residual + pre_ff norm + chunked AG ====
            h2s = {}
            with tc.tile_pool(name="s4", bufs=2) as s4:
                w1_pa_b = s4.tile([128, HID], BF, bufs=1)
                w1_pf_b = s4.tile([128, HID], BF, bufs=1)
                _bcast_row(nc, w1_pa_b, w1_pa, HID)
                _bcast_row(nc, w1_pf_b, w1_pf, HID)
                nscr = s4.tile([128, HID], F32, bufs=1)
                for t in range(2):
                    ao = s4.tile([128, HID], BF, tag="ao", name=f"ao{t}",
                                 bufs=2)
                    nc.gpsimd.dma_start(ao[:], rs_out[t][:])
                    if t == 1:
                        rinv_a = rmsnorm_rinv_act(ao[:], HID, f"pan{t}",
                                                  nscr[:])
                    else:
                        rinv_a = rmsnorm_rinv(ao[:], HID, f"pan{t}")
                    nc.vector.scalar_tensor_tensor(
                        x2_sb[t][:], ao[:], rinv_a[:], w1_pa_b[:],
                        op0=ALU.mult, op1=ALU.mult)
                    nc.vector.tensor_add(x2_sb[t][:], x2_sb[t][:], x_sb[t][:])
                    if t == 1:
                        rinv_f = rmsnorm_rinv_act(x2_sb[t][:], HID, f"pff{t}",
                                                  nscr[:])
                    else:
                        rinv_f = rmsnorm_rinv(x2_sb[t][:], HID, f"pff{t}")
                    h2s[t] = s4.tile([128, HID], BF, tag="h2", name=f"h2_{t}",
                                     bufs=2)
                    nc.vector.scalar_tensor_tensor(
                        h2s[t][:], x2_sb[t][:], rinv_f[:], w1_pf_b[:],
                        op0=ALU.mult, op1=ALU.mult)
                for k in range(KH):
                    for t in range(2):
                        ptr = psP.tile([128, 128], BF, tag="mm", bufs=6,
                                       name=f"s4tr{k}_{t}")
                        nc.tensor.transpose(
                            ptr[:], h2s[t][:, k * 128:(k + 1) * 128], ident[:])
                        hTk = s4.tile([128, 128], BF, tag="hTk",
                                      name=f"hTk{k}_{t}", bufs=4)
                        if (k + t) % 2 == 0:
                            nc.vector.tensor_copy(hTk[:], ptr[:])
                        else:
                            nc.scalar.activation(hTk[:], ptr[:], AF.Copy)
                        nc.sync.dma_start(
                            h2T_in[k * 128:(k + 1) * 128,
                                   t * 128:(t + 1) * 128], hTk[:])
            xpool_cm.__exit__(None, None, None)

            mark('S5')
            # ================= S5: MLP =================
            with tc.tile_pool(name="s5w", bufs=1) as s5w:
                gacc = [s5w.tile([128, S], BF, name=f"gacc{m}")
                        for m in range(MI)]
                uacc = [s5w.tile([128, S], BF, name=f"uacc{m}")
                        for m in range(MI)]
                for g in range(NG):
                    k0, k1 = KSPLIT[g], KSPLIT[g + 1]
                    _coll("AllGather", ALU.bypass,
                          [h2T_in[k0 * 128:k1 * 128, :]],
                          [h2T_full[g][:]])
                with tc.tile_pool(name="s5gu", bufs=2) as s5:
                    for g in range(NG):
                        k0, k1 = KSPLIT[g], KSPLIT[g + 1]
                        kg = k1 - k0
                        h2g = s5.tile([128, 4, NC_, TS], BF, tag="h2g",
                                      name=f"h2g{g}", bufs=2)
                        h2src = h2T_full[g][:].rearrange(
                            "(r k p) t -> r p k t", r=NC_, p=128)
                        for r in range(NC_):
                            nc.gpsimd.dma_start(h2g[:, 0:kg, r, :], h2src[r])
                        h2f = h2g[:].rearrange("p k r t -> p k (r t)")
                        wgm, wum = [], []
                        for hh in range(2):
                            cs = slice(hh * 640, (hh + 1) * 640)
                            wg_h = s5.tile([128, 4, 640], BF, tag="wg",
                                           name=f"wgm{g}_{hh}", bufs=2)
                            wu_h = s5.tile([128, 4, 640], BF, tag="wu",
                                           name=f"wum{g}_{hh}", bufs=2)
                            nc.sync.dma_start(
                                wg_h[:, 0:kg, :],
                                wg_t.ap()[k0:k1].rearrange(
                                    "k p n -> p k n")[:, :, cs])
                            nc.sync.dma_start(
                                wu_h[:, 0:kg, :],
                                wu_t.ap()[k0:k1].rearrange(
                                    "k p n -> p k n")[:, :, cs])
                            wgm.append(wg_h)
                            wum.append(wu_h)
                        for m in range(MI):
                            hh, mo = m // 5, (m % 5) * 128
                            if g == NG - 1 and m > 0:
                                mm_ = m - 1
                                gt = s5.tile([128, S], BF, tag="gt",
                                             name=f"gt{mm_}", bufs=2)
                                nc.scalar.activation(gt[:], gacc[mm_][:],
                                                     AF.Gelu_apprx_tanh)
                                nc.gpsimd.tensor_mul(uacc[mm_][:], gt[:],
                                                     uacc[mm_][:])
                            for rp in range(4):
                                cs = slice(rp * 512, (rp + 1) * 512)
                                pg = psP.tile([128, 512], F32, tag="mm",
                                              bufs=6, name=f"pg{g}{m}{rp}")
                                pu = psP.tile([128, 512], F32, tag="mm",
                                              bufs=6, name=f"pu{g}{m}{rp}")
                                for k in range(kg):
                                    st, sp = (k == 0), (k == kg - 1)
                                    nc.tensor.matmul(
                                        pg[:], wgm[hh][:, k, mo:mo + 128],
                                        h2f[:, k, cs], start=st, stop=sp)
                                    nc.tensor.matmul(
                                        pu[:], wum[hh][:, k, mo:mo + 128],
                                        h2f[:, k, cs], start=st, stop=sp)
                                if g == 0:
                                    if (m + rp) % 2 == 0:
                                        nc.scalar.activation(gacc[m][:, cs],
                                                             pg[:], AF.Copy)
                                        nc.vector.tensor_copy(uacc[m][:, cs],
                                                              pu[:])
                                    else:
                                        nc.vector.tensor_copy(gacc[m][:, cs],
                                                              pg[:])
                                        nc.scalar.activation(uacc[m][:, cs],
                                                             pu[:], AF.Copy)
                                else:
                                    nc.vector.tensor_add(gacc[m][:, cs],
                                                         gacc[m][:, cs],
                                                         pg[:])
                                    nc.vector.tensor_add(uacc[m][:, cs],
                                                         uacc[m][:, cs],
                                                         pu[:])
                    # last m's act (the rest were interleaved into pass 3)
                    gt = s5.tile([128, S], BF, tag="gt", name=f"gt{MI-1}",
                                 bufs=2)
                    nc.scalar.activation(gt[:], gacc[MI - 1][:],
                                         AF.Gelu_apprx_tanh)
                    nc.gpsimd.tensor_mul(uacc[MI - 1][:], gt[:],
                                         uacc[MI - 1][:])
                # down proj, striped over token-tile parity
                with tc.tile_pool(name="s5d", bufs=2) as s5d:
                    for j in range(2):
                        for n in range(5):
                            wdn = s5d.tile([128, MI, 512], BF, tag="wdn",
                                           name=f"wdn{j}_{n}", bufs=2)
                            nc.sync.dma_start(
                                wdn[:], wd_t.ap()[n].rearrange("m p c -> p m c"))
                            for blk in range(NC_):
                                tt = 2 * blk + j
                                pd = psP.tile([128, 512], F32, tag="mm",
                                              bufs=6, name=f"pd{tt}_{n}")
                                for i in range(MI):
                                    nc.tensor.matmul(
                                        pd[:],
                                        uacc[i][:, tt * 128:(tt + 1) * 128],
                                        wdn[:, i, :], start=(i == 0),
                                        stop=(i == MI - 1))
                                dcp = s5d.tile([128, 512], BF, tag="dcp",
                                               name=f"dcp{tt}_{n}", bufs=4)
                                if n % 2 == 0:
                                    nc.vector.tensor_copy(dcp[:], pd[:])
                                else:
                                    nc.scalar.activation(dcp[:], pd[:],
                                                         AF.Copy)
                                nc.sync.dma_start(
                                    rs2_in[j][:][blk * 128:(blk + 1) * 128,
                                                 n * 512:(n + 1) * 512],
                                    dcp[:])
                        _coll("ReduceScatter", ALU.add, [rs2_in[j][:]],
                              [rs2_out[j][:]])

            mark('S6')
            # ============ S6: post_ff norm + residual ============
            with tc.tile_pool(name="s6", bufs=2) as s6:
                w1_po_b = s6.tile([128, HID], F32, bufs=1)
                _bcast_row(nc, w1_po_b, w1_po, HID)
                nscr6 = s6.tile([128, HID], F32, bufs=1)
                for t in range(2):
                    mlp16 = s6.tile([128, HID], BF, tag="mlp", name=f"mlp{t}",
                                    bufs=2)
                    nc.gpsimd.dma_start(mlp16[:], rs2_out[t][:])
                    if t == 1:
                        rinv_o = rmsnorm_rinv_act(mlp16[:], HID, f"pon{t}",
                                                  nscr6[:])
                    else:
                        rinv_o = rmsnorm_rinv(mlp16[:], HID, f"pon{t}")
                    o32 = s6.tile([128, HID], F32, tag="o32", name=f"o32_{t}",
                                  bufs=2)
                    nc.vector.scalar_tensor_tensor(o32[:], mlp16[:], rinv_o[:],
                                                   w1_po_b[:], op0=ALU.mult,
                                                   op1=ALU.mult)
                    nc.vector.tensor_add(o32[:], o32[:], x2_sb[t][:])
                    nc.sync.dma_start(out_shard.ap()[t * 128:(t + 1) * 128, :],
                                      o32[:])
            x2pool_cm.__exit__(None, None, None)

    nc.compile()
    return nc


_NC_CACHE = None


def _get_nc():
    global _NC_CACHE
    if _NC_CACHE is None:
        _NC_CACHE = build_nc()
    return _NC_CACHE


def make_in_maps(hidden_states, position_ids, wq, wk, wv, wo, q_ln_w, k_ln_w,
                 in_ln_w, post_attn_ln_w, pre_ff_ln_w, post_ff_ln_w,
                 w_gate, w_up, w_down):
    bf16 = ml_dtypes.bfloat16
    f32 = np.float32
    x = np.asarray(hidden_states, f32).reshape(S, HID)
    pos = np.asarray(position_ids).reshape(S).astype(np.float64)

    inv_freq = 1.0 / (BASE ** (np.arange(0, HD, 2, dtype=np.float64) / HD))
    freqs = pos[:, None] * inv_freq[None, :]
    emb = np.concatenate([freqs, freqs], axis=1)
    cos = np.cos(emb).astype(f32)
    sin = np.sin(emb).astype(f32)
    w1q = 1.0 + np.asarray(q_ln_w, f32)
    w1k = 1.0 + np.asarray(k_ln_w, f32)

    def rope_tabs(w1):
        w1sw = np.concatenate([w1[HALF:], w1[:HALF]])
        sgn = np.concatenate([-np.ones(HALF, f32), np.ones(HALF, f32)])
        return ((cos * w1[None, :]).astype(bf16),
                (sin * (w1sw * sgn)[None, :]).astype(bf16))

    cqw_np, sqw_np = rope_tabs(w1q)
    ckw_np, skw_np = rope_tabs(w1k)

    # x^T replicated, split in 4 column (token) groups for early DMA
    xT = np.ascontiguousarray(x.T.astype(bf16))          # [HID, S]
    xT4 = np.ascontiguousarray(
        xT.reshape(HID, 4, S // 4).transpose(1, 0, 2))   # [4, HID, S/4]

    # fold (1 + in_ln_w) into the QKV weights
    fold = (1.0 + np.asarray(in_ln_w, f32))[:, None]
    wq_f = (np.asarray(wq, f32) * fold).reshape(HID, NH, HD)
    wk_f = (np.asarray(wk, f32) * fold).reshape(HID, NKV, HD)
    wv_f = (np.asarray(wv, f32) * fold).reshape(HID, NKV, HD)
    wo_r = np.asarray(wo, f32).reshape(NH, HD, HID)
    wg_r = np.asarray(w_gate, f32).reshape(HID, NC_, INTER // NC_)
    wu_r = np.asarray(w_up, f32).reshape(HID, NC_, INTER // NC_)
    wd_r = np.asarray(w_down, f32).reshape(NC_, INTER // NC_, HID)

    common = {
        "xT4": xT4,
        "w1_pa": (1.0 + np.asarray(post_attn_ln_w, f32)).astype(bf16),
        "w1_pf": (1.0 + np.asarray(pre_ff_ln_w, f32)).astype(bf16),
        "w1_po": 1.0 + np.asarray(post_ff_ln_w, f32),
        "cqw": cqw_np, "sqw": sqw_np, "ckw": ckw_np, "skw": skw_np,
    }
    in_maps = []
    for c in range(NC_):
        g = c // (NH // NKV)
        wqk = np.concatenate([wq_f[:, c, :], wk_f[:, g, :]], axis=1)
        in_maps.append({
            "x_shard": np.ascontiguousarray(x[c * TS:(c + 1) * TS]),
            "wqk_c": np.ascontiguousarray(wqk).astype(bf16),
            "wv_c": np.ascontiguousarray(wv_f[:, g, :]).astype(bf16),
            "wo_c": np.ascontiguousarray(wo_r[c]).astype(bf16),
            # wg_t/wu_t: [KH, 128, 1280] so each k-chunk row is contiguous
            "wg_t": np.ascontiguousarray(
                wg_r[:, c, :].reshape(KH, 128, MI * 128)).astype(bf16),
            "wu_t": np.ascontiguousarray(
                wu_r[:, c, :].reshape(KH, 128, MI * 128)).astype(bf16),
            # wd_t: [5, MI, 128, 512]: (col-chunk, m, inter-row, col)
            "wd_t": np.ascontiguousarray(
                wd_r[c].reshape(MI, 128, 5, 512).transpose(2, 0, 1, 3)
            ).astype(bf16),
            **common,
        })
    return in_maps


def kernel(**inputs):
    in_maps = make_in_maps(**inputs)
    nc = _get_nc()
    res = run_bass_kernel_spmd(nc, in_maps, core_ids=list(range(NC_)))
    out = np.concatenate([res.results[c]["out_shard"] for c in range(NC_)], axis=0)
    return out.reshape(1, S, HID).astype(np.float32)
